# revision 1
# baseline (speedup 1.0000x reference)
"""Butterfly permuter kernel for Trainium2 (8 NeuronCores, SPMD data-parallel).

The reference applies 10 butterfly rotation stages along the feature axis
(dim=1024) of x [16384, 1024].  Each row is transformed independently, and the
10 stages compose into a single dense 1024x1024 orthogonal matrix R with
y_rows = x_rows @ R.  We compute R on the host in float64 from `angles`, then
run a tiled matmul on each core:

  per core: x_shard [2048, 1024]
  - DMA x in 2 MiB megatiles [128 part, 4096] (4 row-subtiles of 128 tokens)
  - PE-transpose each [128 tok, 128 dim] block (float32r, via identity) to get
    X^T blocks (contraction dim on partitions), evacuate PSUM->SBUF on ScalarE
  - 16 accumulating float32r matmuls per subtile: psum_y[jh] += XT_kb^T @ R_kb
    (float32r streams 1 cycle/row at N=512 - full PE rate, ~fp32 storage)
  - evacuate y PSUM->SBUF on VectorE, DMA out 2 MiB megatiles

Inputs arrive full-size; sharding is across the token axis (2048 rows/core).
"""

import numpy as np

import concourse.bass as bass
import concourse.mybir as mybir
import concourse.tile as tile
from concourse import bacc
from concourse.bass_utils import run_bass_kernel_spmd

N_CORES = 8
DIM = 1024
NUM_STAGES = 10
N_TOKENS = 16384
TOK_PER_CORE = N_TOKENS // N_CORES  # 2048
SUB = 128  # tokens per subtile (partition dim)
SUBTILES_PER_MEGA = 4
MEGA_ROWS = SUB * SUBTILES_PER_MEGA  # 512 tokens per DMA megatile
N_MEGA = TOK_PER_CORE // MEGA_ROWS  # 4
KB = DIM // 128  # 8 contraction blocks

F32 = mybir.dt.float32
F32R = mybir.dt.float32r


def compose_transform(angles: np.ndarray) -> np.ndarray:
    """Compose the 10 butterfly stages into R (float32) with y = x @ R."""
    y = np.eye(DIM, dtype=np.float64)
    a = np.asarray(angles, dtype=np.float64)
    for s in range(NUM_STAGES):
        span = 2 ** (s + 1)
        half = span // 2
        y = y.reshape(-1, DIM // span, span)
        left, right = y[..., :half], y[..., half:]
        th = a[s].reshape(1, DIM // span, half)
        c, sn = np.cos(th), np.sin(th)
        y = np.concatenate([c * left + sn * right, -sn * left + c * right], -1)
        y = y.reshape(-1, DIM)
    # row t of y is transform(e_t), so transform(x) = x @ y
    return np.ascontiguousarray(y, dtype=np.float32)


def build_bass(reps: int = 1):
    """reps>1 repeats the whole pipeline in one NEFF (for marginal timing)."""
    nc = bacc.Bacc(None, target_bir_lowering=False)
    x = nc.dram_tensor("x", [TOK_PER_CORE, DIM], F32, kind="ExternalInput")
    w = nc.dram_tensor("w", [DIM, DIM], F32, kind="ExternalInput")
    ident = nc.dram_tensor("ident", [128, 128], F32, kind="ExternalInput")
    y = nc.dram_tensor("y", [TOK_PER_CORE, DIM], F32, kind="ExternalOutput")

    n_sub = N_MEGA * SUBTILES_PER_MEGA  # 16 subtiles of 128 tokens

    # Variable-size DMA chunking (in units of 128-token subtiles): small
    # chunks at the start for a fast pipeline ramp, small at the end for a
    # short drain; 2-subtile (1 MiB) chunks in steady state.
    in_chunks = [1, 1, 2, 2, 2, 2, 2, 2, 2]
    out_chunks = [2, 2, 2, 2, 2, 2, 2, 1, 1]
    assert sum(in_chunks) == n_sub and sum(out_chunks) == n_sub
    in_start = [sum(in_chunks[:i]) for i in range(len(in_chunks))]
    out_start = [sum(out_chunks[:i]) for i in range(len(out_chunks))]
    sub_to_in_chunk = {}
    for ci, (st, ln) in enumerate(zip(in_start, in_chunks)):
        for s in range(st, st + ln):
            sub_to_in_chunk[s] = ci
    sub_to_out_chunk = {}
    for ci, (st, ln) in enumerate(zip(out_start, out_chunks)):
        for s in range(st, st + ln):
            sub_to_out_chunk[s] = ci

    with tile.TileContext(nc) as tc:
        with (
            tc.tile_pool(name="const", bufs=1) as const_pool,
            tc.tile_pool(name="wstage", bufs=3) as wstage_pool,
            tc.tile_pool(name="xin", bufs=3) as xin_pool,
            tc.tile_pool(name="xt", bufs=5) as xt_pool,
            tc.tile_pool(name="yout", bufs=3) as yout_pool,
            tc.tile_pool(name="pst", bufs=4, space="PSUM") as pst_pool,
            tc.tile_pool(name="psy", bufs=4, space="PSUM") as psy_pool,
        ):
            # identity goes via the SWDGE ring; the SP ring starts with the
            # first x chunk; W streams in behind it.
            ident_sb = const_pool.tile([128, 128], F32, name="ident_sb")
            nc.gpsimd.dma_start(ident_sb[:], ident[:])

            x_tiles = [None] * len(in_chunks)  # chunk idx -> (tile, start_sub)
            y_tiles = [None] * len(out_chunks)

            def load_chunk(ci):
                st, ln = in_start[ci], in_chunks[ci]
                x_tile = xin_pool.tile([128, ln * DIM], F32, name="x_chunk",
                                       tag="x_chunk",
                                       padded_shape=[128, 2 * DIM])
                r0 = st * SUB
                nc.sync.dma_start(
                    x_tile[:, : ln * DIM].rearrange("p (s c) -> p s c", c=DIM),
                    x[r0 : r0 + ln * SUB, :].rearrange("(s p) c -> p s c", p=128),
                )
                x_tiles[ci] = x_tile

            load_chunk(0)
            first_load_done = True

            # W: DMA [jh][kb] blocks of [128,512] (j-half-major so the first
            # 2 MiB unblocks the first matmul group) on the ACT HWDGE ring,
            # then round fp32 -> f32r on DVE (walrus requires f32r matmul
            # inputs to come from a rounding instruction).
            w_sbr = const_pool.tile([128, KB * DIM], F32R, name="w_sbr")

            def w_off(jh, kb):
                return (jh * KB + kb) * 512

            for jh in range(2):
                for kb in range(KB):
                    w_stage = wstage_pool.tile([128, 512], F32, name="w_stage",
                                               tag="w_stage")
                    nc.sync.dma_start(
                        w_stage[:],
                        w[kb * 128 : (kb + 1) * 128, jh * 512 : (jh + 1) * 512],
                    )
                    off = w_off(jh, kb)
                    nc.vector.tensor_copy(w_sbr[:, off : off + 512], w_stage[:])

            xts = [None] * n_sub

            def emit_transpose(s):
                ci = sub_to_in_chunk[s]
                xcol = (s - in_start[ci]) * DIM
                x_tile = x_tiles[ci]
                ps_t0 = pst_pool.tile([128, 512], F32, name="ps_t0", tag="ps_t")
                ps_t1 = pst_pool.tile([128, 512], F32, name="ps_t1", tag="ps_t")
                for kb in range(KB):
                    dst = ps_t0 if kb < 4 else ps_t1
                    j = (kb % 4) * 128
                    nc.tensor.transpose(
                        dst[:, j : j + 128],
                        x_tile[:, xcol + kb * 128 : xcol + (kb + 1) * 128],
                        ident_sb,
                    )
                xt = xt_pool.tile([128, DIM], F32R, name="xt", tag="xt")
                nc.scalar.copy(xt[:, :512], ps_t0[:])
                nc.scalar.copy(xt[:, 512:], ps_t1[:])
                xts[s] = xt

            def emit_matmul(s, jh):
                co = sub_to_out_chunk[s]
                st, ln = out_start[co], out_chunks[co]
                if s == st and jh == 0:
                    y_tiles[co] = yout_pool.tile(
                        [128, ln * DIM], F32, name="y_chunk", tag="y_chunk",
                        padded_shape=[128, 2 * DIM],
                    )
                y_tile = y_tiles[co]
                ycol = (s - st) * DIM + jh * 512
                xt = xts[s]
                ps_y = psy_pool.tile([128, 512], F32, name="ps_y", tag="ps_y")
                for kb in range(KB):
                    off = (jh * KB + kb) * 512
                    nc.tensor.matmul(
                        ps_y[:],
                        xt[:, kb * 128 : (kb + 1) * 128],
                        w_sbr[:, off : off + 512],
                        start=(kb == 0),
                        stop=(kb == KB - 1),
                    )
                nc.vector.tensor_copy(y_tile[:, ycol : ycol + 512], ps_y[:])
                if s == st + ln - 1 and jh == 1:
                    r0 = st * SUB
                    # y stores go out on the ACT HWDGE ring so they don't
                    # queue ahead of later x loads on the SP ring.
                    nc.scalar.dma_start(
                        y[r0 : r0 + ln * SUB, :].rearrange("(s p) c -> p s c", p=128),
                        y_tile[:, : ln * DIM].rearrange("p (s c) -> p s c", c=DIM),
                    )

            # Skewed software pipeline: transposes run one subtile ahead of
            # the matmuls so the PE never waits on the ScalarE PSUM->SBUF
            # evacuation of its own transpose outputs.
            # Transposes run two subtiles ahead of the matmuls (more PE
            # runway while W streams in), and j-halves are staggered one
            # subtile apart: MM(s, jh0) then MM(s-1, jh1), so subtile 0's
            # jh1 group (which needs the second half of W) doesn't stall
            # the in-order PE stream at startup.
            SKEW = 2
            for _rep in range(reps):
                if not first_load_done:
                    load_chunk(0)
                first_load_done = False
                for p in range(min(SKEW, n_sub)):
                    ci = sub_to_in_chunk[p]
                    if p == in_start[ci] and p > 0:
                        load_chunk(ci)
                    emit_transpose(p)
                for s in range(n_sub):
                    nxt = s + SKEW
                    if nxt < n_sub:
                        ci = sub_to_in_chunk[nxt]
                        if nxt == in_start[ci]:
                            load_chunk(ci)
                        emit_transpose(nxt)
                    emit_matmul(s, 0)
                    if s >= 1:
                        emit_matmul(s - 1, 1)
                emit_matmul(n_sub - 1, 1)
    nc.compile()
    return nc


_NC_CACHE = None


def _get_nc():
    global _NC_CACHE
    if _NC_CACHE is None:
        _NC_CACHE = build_bass()
    return _NC_CACHE


def run(x: np.ndarray, angles: np.ndarray, trace: bool = False):
    """Run on 8 cores; returns (y_full, BassKernelResults)."""
    x = np.ascontiguousarray(np.asarray(x, dtype=np.float32))
    w = compose_transform(angles)
    ident = np.eye(128, dtype=np.float32)
    nc = _get_nc()
    in_maps = []
    for c in range(N_CORES):
        in_maps.append(
            {
                "x": x[c * TOK_PER_CORE : (c + 1) * TOK_PER_CORE],
                "w": w,
                "ident": ident,
            }
        )
    res = run_bass_kernel_spmd(
        nc, in_maps, core_ids=list(range(N_CORES)), trace=trace
    )
    y = np.concatenate([res.results[c]["y"] for c in range(N_CORES)], axis=0)
    return y, res


def kernel(x: np.ndarray, angles: np.ndarray) -> np.ndarray:
    y, _ = run(x, angles, trace=False)
    return y



# revision 2
# speedup vs baseline: 1.6597x; 1.6597x over previous
"""Butterfly permuter kernel for Trainium2 (8 NeuronCores, SPMD data-parallel).

The reference applies 10 butterfly rotation stages along the feature axis
(dim=1024) of x [16384, 1024].  Stages 1-9 act within the two 512-wide
feature halves, so they compose into a block-diagonal matrix
R9 = blockdiag(A0, A1) with two dense 512x512 blocks (computed on the host
in float64).  Stage 10 couples feature f with f+512 through an elementwise
Givens rotation whose cos/sin vary along the feature axis.

Per core (x shard [2048, 1024], 16 subtiles of 128 tokens):
  - DMA x in megatiles, PE-transpose each [128 tok, 128 feat] block via
    identity (PSUM), evacuate to SBUF on ScalarE as f32r
  - z halves: ps_l = sum_{kb<4} xt_kb^T @ A0_kb, ps_r = sum_{kb>=4} ... -
    8 accumulating f32r matmuls of N=512 per subtile (half the PE work of
    the dense 1024x1024 formulation)
  - stage 10 fused into PSUM evacuation: 4 DVE multiplies against
    broadcast cos/sin tiles + 2 GpSimd add/sub produce
    y_l = c*z_l + s*z_r, y_r = c*z_r - s*z_l directly in the y SBUF tile
  - DMA y out in megatiles

Engine budget per rep per core: PE ~41 us, DVE ~17 us, ScalarE ~14 us,
GpSimd ~14 us, DMA 16 MiB ~47 us -> memory-bound as targeted.
"""

import numpy as np

import concourse.bass as bass
import concourse.mybir as mybir
import concourse.tile as tile
from concourse import bacc
from concourse.bass_utils import run_bass_kernel_spmd

N_CORES = 8
DIM = 1024
HALF = DIM // 2
NUM_STAGES = 10
N_TOKENS = 16384
TOK_PER_CORE = N_TOKENS // N_CORES  # 2048
SUB = 128  # tokens per subtile (partition dim)
KB = DIM // 128  # 8 feature blocks

F32 = mybir.dt.float32
F32R = mybir.dt.float32r
MULT = mybir.AluOpType.mult
ADD = mybir.AluOpType.add
SUBTRACT = mybir.AluOpType.subtract


def _compose(angles: np.ndarray, n_stages: int) -> np.ndarray:
    """Compose the first n_stages butterfly stages: y = x @ R."""
    y = np.eye(DIM, dtype=np.float64)
    a = np.asarray(angles, dtype=np.float64)
    for s in range(n_stages):
        span = 2 ** (s + 1)
        half = span // 2
        y = y.reshape(-1, DIM // span, span)
        left, right = y[..., :half], y[..., half:]
        th = a[s].reshape(1, DIM // span, half)
        c, sn = np.cos(th), np.sin(th)
        y = np.concatenate([c * left + sn * right, -sn * left + c * right], -1)
        y = y.reshape(-1, DIM)
    return y


def host_inputs(angles: np.ndarray) -> dict:
    """Per-core constant inputs: stacked R9 blocks + broadcast cos/sin."""
    r9 = _compose(angles, NUM_STAGES - 1)  # block-diagonal: two 512x512
    w9 = np.concatenate([r9[:HALF, :HALF], r9[HALF:, HALF:]], axis=0)
    a9 = np.asarray(angles, dtype=np.float64)[NUM_STAGES - 1]
    c = np.cos(a9).astype(np.float32)
    s = np.sin(a9).astype(np.float32)
    cs = np.concatenate(
        [
            np.broadcast_to(c, (128, HALF)),
            np.broadcast_to(s, (128, HALF)),
        ],
        axis=0,
    )
    return {
        "w": np.ascontiguousarray(w9, dtype=np.float32),
        "cs": np.ascontiguousarray(cs, dtype=np.float32),
        "ident": np.eye(128, dtype=np.float32),
    }


def build_bass(reps: int = 1):
    """reps>1 repeats the whole pipeline in one NEFF (for marginal timing)."""
    nc = bacc.Bacc(None, target_bir_lowering=False)
    x = nc.dram_tensor("x", [TOK_PER_CORE, DIM], F32, kind="ExternalInput")
    w = nc.dram_tensor("w", [DIM, HALF], F32, kind="ExternalInput")
    cs = nc.dram_tensor("cs", [256, HALF], F32, kind="ExternalInput")
    ident = nc.dram_tensor("ident", [128, 128], F32, kind="ExternalInput")
    y = nc.dram_tensor("y", [TOK_PER_CORE, DIM], F32, kind="ExternalOutput")

    n_sub = TOK_PER_CORE // SUB  # 16 subtiles of 128 tokens

    # Variable-size DMA chunking (in units of 128-token subtiles): small
    # chunks at the start for a fast pipeline ramp, small at the end for a
    # short drain; 2-subtile (1 MiB) chunks in steady state.
    in_chunks = [1, 1, 2, 2, 2, 2, 2, 2, 2]
    out_chunks = [2, 2, 2, 2, 2, 2, 2, 1, 1]
    assert sum(in_chunks) == n_sub and sum(out_chunks) == n_sub
    in_start = [sum(in_chunks[:i]) for i in range(len(in_chunks))]
    out_start = [sum(out_chunks[:i]) for i in range(len(out_chunks))]
    sub_to_in_chunk = {}
    for ci, (st, ln) in enumerate(zip(in_start, in_chunks)):
        for s in range(st, st + ln):
            sub_to_in_chunk[s] = ci
    sub_to_out_chunk = {}
    for ci, (st, ln) in enumerate(zip(out_start, out_chunks)):
        for s in range(st, st + ln):
            sub_to_out_chunk[s] = ci

    with tile.TileContext(nc) as tc:
        with (
            tc.tile_pool(name="const", bufs=1) as const_pool,
            tc.tile_pool(name="wstage", bufs=3) as wstage_pool,
            tc.tile_pool(name="xin", bufs=3) as xin_pool,
            tc.tile_pool(name="xt", bufs=5) as xt_pool,
            tc.tile_pool(name="tmp", bufs=8) as tmp_pool,
            tc.tile_pool(name="yout", bufs=3) as yout_pool,
            tc.tile_pool(name="pst", bufs=4, space="PSUM") as pst_pool,
            tc.tile_pool(name="psy", bufs=4, space="PSUM") as psy_pool,
        ):
            # identity goes via the SWDGE ring; the SP ring starts with the
            # first x chunk; W streams in behind it.
            ident_sb = const_pool.tile([128, 128], F32, name="ident_sb")
            nc.gpsimd.dma_start(ident_sb[:], ident[:])

            c_sb = const_pool.tile([128, HALF], F32, name="c_sb")
            s_sb = const_pool.tile([128, HALF], F32, name="s_sb")
            nc.gpsimd.dma_start(c_sb[:], cs[0:128, :])
            nc.gpsimd.dma_start(s_sb[:], cs[128:256, :])

            x_tiles = [None] * len(in_chunks)  # chunk idx -> tile
            y_tiles = [None] * len(out_chunks)

            def load_chunk(ci):
                st, ln = in_start[ci], in_chunks[ci]
                x_tile = xin_pool.tile([128, ln * DIM], F32, name="x_chunk",
                                       tag="x_chunk",
                                       padded_shape=[128, 2 * DIM])
                r0 = st * SUB
                nc.sync.dma_start(
                    x_tile[:, : ln * DIM].rearrange("p (s c) -> p s c", c=DIM),
                    x[r0 : r0 + ln * SUB, :].rearrange("(s p) c -> p s c", p=128),
                )
                x_tiles[ci] = x_tile

            load_chunk(0)
            first_load_done = True

            # W: DMA the 8 stacked [128, 512] blocks of R9 on the SP ring,
            # then round fp32 -> f32r on DVE (walrus requires f32r matmul
            # inputs to come from a rounding instruction).
            w_sbr = const_pool.tile([128, KB * HALF], F32R, name="w_sbr")
            for kb in range(KB):
                w_stage = wstage_pool.tile([128, HALF], F32, name="w_stage",
                                           tag="w_stage")
                nc.sync.dma_start(
                    w_stage[:], w[kb * 128 : (kb + 1) * 128, :]
                )
                off = kb * HALF
                nc.vector.tensor_copy(w_sbr[:, off : off + HALF], w_stage[:])

            xts = [None] * n_sub

            def emit_transpose(s):
                ci = sub_to_in_chunk[s]
                xcol = (s - in_start[ci]) * DIM
                x_tile = x_tiles[ci]
                ps_t0 = pst_pool.tile([128, 512], F32, name="ps_t0", tag="ps_t")
                ps_t1 = pst_pool.tile([128, 512], F32, name="ps_t1", tag="ps_t")
                for kb in range(KB):
                    dst = ps_t0 if kb < 4 else ps_t1
                    j = (kb % 4) * 128
                    nc.tensor.transpose(
                        dst[:, j : j + 128],
                        x_tile[:, xcol + kb * 128 : xcol + (kb + 1) * 128],
                        ident_sb,
                    )
                xt = xt_pool.tile([128, DIM], F32R, name="xt", tag="xt")
                nc.scalar.copy(xt[:, :512], ps_t0[:])
                nc.scalar.copy(xt[:, 512:], ps_t1[:])
                xts[s] = xt

            def emit_matmul(s):
                co = sub_to_out_chunk[s]
                st, ln = out_start[co], out_chunks[co]
                if s == st:
                    y_tiles[co] = yout_pool.tile(
                        [128, ln * DIM], F32, name="y_chunk", tag="y_chunk",
                        padded_shape=[128, 2 * DIM],
                    )
                y_tile = y_tiles[co]
                ycol = (s - st) * DIM
                xt = xts[s]
                ps_l = psy_pool.tile([128, HALF], F32, name="ps_l", tag="ps_y")
                ps_r = psy_pool.tile([128, HALF], F32, name="ps_r", tag="ps_y")
                for kb in range(4):
                    nc.tensor.matmul(
                        ps_l[:],
                        xt[:, kb * 128 : (kb + 1) * 128],
                        w_sbr[:, kb * HALF : (kb + 1) * HALF],
                        start=(kb == 0),
                        stop=(kb == 3),
                    )
                for kb in range(4, 8):
                    nc.tensor.matmul(
                        ps_r[:],
                        xt[:, kb * 128 : (kb + 1) * 128],
                        w_sbr[:, kb * HALF : (kb + 1) * HALF],
                        start=(kb == 4),
                        stop=(kb == 7),
                    )
                # stage 10: y_l = c*z_l + s*z_r, y_r = c*z_r - s*z_l.
                # DVE does the 4 PSUM-reading multiplies, GpSimd the
                # SBUF-only combines straight into the y chunk tile.
                t1 = tmp_pool.tile([128, HALF], F32, name="t1", tag="tmp")
                u1 = tmp_pool.tile([128, HALF], F32, name="u1", tag="tmp")
                t2 = tmp_pool.tile([128, HALF], F32, name="t2", tag="tmp")
                u2 = tmp_pool.tile([128, HALF], F32, name="u2", tag="tmp")
                nc.vector.tensor_tensor(t1[:], ps_l[:], c_sb[:], MULT)
                nc.vector.tensor_tensor(u1[:], ps_r[:], s_sb[:], MULT)
                nc.vector.tensor_tensor(u2[:], ps_l[:], s_sb[:], MULT)
                nc.vector.tensor_tensor(t2[:], ps_r[:], c_sb[:], MULT)
                nc.gpsimd.tensor_tensor(
                    y_tile[:, ycol : ycol + HALF], t1[:], u1[:], ADD
                )
                nc.gpsimd.tensor_tensor(
                    y_tile[:, ycol + HALF : ycol + DIM], t2[:], u2[:], SUBTRACT
                )
                if s == st + ln - 1:
                    r0 = st * SUB
                    # y stores go out on the ACT HWDGE ring so they don't
                    # queue ahead of later x loads on the SP ring.
                    nc.scalar.dma_start(
                        y[r0 : r0 + ln * SUB, :].rearrange("(s p) c -> p s c", p=128),
                        y_tile[:, : ln * DIM].rearrange("p (s c) -> p s c", c=DIM),
                    )

            # Skewed software pipeline: transposes run two subtiles ahead of
            # the matmuls so the PE never waits on the ScalarE PSUM->SBUF
            # evacuation of its own transpose outputs.
            SKEW = 2
            for _rep in range(reps):
                if not first_load_done:
                    load_chunk(0)
                first_load_done = False
                for p in range(min(SKEW, n_sub)):
                    ci = sub_to_in_chunk[p]
                    if p == in_start[ci] and p > 0:
                        load_chunk(ci)
                    emit_transpose(p)
                for s in range(n_sub):
                    nxt = s + SKEW
                    if nxt < n_sub:
                        ci = sub_to_in_chunk[nxt]
                        if nxt == in_start[ci]:
                            load_chunk(ci)
                        emit_transpose(nxt)
                    emit_matmul(s)
    nc.compile()
    return nc


_NC_CACHE = None


def _get_nc():
    global _NC_CACHE
    if _NC_CACHE is None:
        _NC_CACHE = build_bass()
    return _NC_CACHE


def run(x: np.ndarray, angles: np.ndarray, trace: bool = False):
    """Run on 8 cores; returns (y_full, BassKernelResults)."""
    x = np.ascontiguousarray(np.asarray(x, dtype=np.float32))
    consts = host_inputs(angles)
    nc = _get_nc()
    in_maps = []
    for c in range(N_CORES):
        in_maps.append(
            {"x": x[c * TOK_PER_CORE : (c + 1) * TOK_PER_CORE], **consts}
        )
    res = run_bass_kernel_spmd(
        nc, in_maps, core_ids=list(range(N_CORES)), trace=trace
    )
    y = np.concatenate([res.results[c]["y"] for c in range(N_CORES)], axis=0)
    return y, res


def kernel(x: np.ndarray, angles: np.ndarray) -> np.ndarray:
    y, _ = run(x, angles, trace=False)
    return y


# revision 9
# speedup vs baseline: 1.7921x; 1.0798x over previous
"""Butterfly permuter kernel for Trainium2 (8 NeuronCores, SPMD data-parallel).

The reference applies 10 butterfly rotation stages along the feature axis
(dim=1024) of x [16384, 1024].  Stages 1-9 act within the two 512-wide
feature halves, so they compose into a block-diagonal matrix
R9 = blockdiag(A0, A1) with two dense 512x512 blocks (computed on the host
in float64).  Stage 10 couples feature f with f+512 through an elementwise
Givens rotation whose cos/sin vary along the feature axis.

Per core (x shard [2048, 1024], 16 subtiles of 128 tokens):
  - DMA x in megatiles, PE-transpose each [128 tok, 128 feat] block via
    identity (PSUM), evacuate to SBUF on ScalarE as f32r
  - z halves: ps_l = sum_{kb<4} xt_kb^T @ A0_kb, ps_r = sum_{kb>=4} ... -
    8 accumulating f32r matmuls of N=512 per subtile (half the PE work of
    the dense 1024x1024 formulation)
  - stage 10 fused into PSUM evacuation: 4 DVE multiplies against
    broadcast cos/sin tiles + 2 GpSimd add/sub produce
    y_l = c*z_l + s*z_r, y_r = c*z_r - s*z_l directly in the y SBUF tile
  - DMA y out in megatiles

Engine budget per rep per core: PE ~41 us, DVE ~17 us, ScalarE ~14 us,
GpSimd ~14 us, DMA 16 MiB ~47 us -> memory-bound as targeted.
"""

import numpy as np

import concourse.bass as bass
import concourse.mybir as mybir
import concourse.tile as tile
from concourse import bacc
from concourse.bass_utils import run_bass_kernel_spmd

N_CORES = 8
DIM = 1024
HALF = DIM // 2
NUM_STAGES = 10
N_TOKENS = 16384
TOK_PER_CORE = N_TOKENS // N_CORES  # 2048
SUB = 128  # tokens per subtile (partition dim)
KB = DIM // 128  # 8 feature blocks

F32 = mybir.dt.float32
F32R = mybir.dt.float32r
MULT = mybir.AluOpType.mult
ADD = mybir.AluOpType.add
SUBTRACT = mybir.AluOpType.subtract


def _compose(angles: np.ndarray, n_stages: int) -> np.ndarray:
    """Compose the first n_stages butterfly stages: y = x @ R."""
    y = np.eye(DIM, dtype=np.float64)
    a = np.asarray(angles, dtype=np.float64)
    for s in range(n_stages):
        span = 2 ** (s + 1)
        half = span // 2
        y = y.reshape(-1, DIM // span, span)
        left, right = y[..., :half], y[..., half:]
        th = a[s].reshape(1, DIM // span, half)
        c, sn = np.cos(th), np.sin(th)
        y = np.concatenate([c * left + sn * right, -sn * left + c * right], -1)
        y = y.reshape(-1, DIM)
    return y


def host_inputs(angles: np.ndarray) -> dict:
    """Per-core constant inputs: stacked R9 blocks + broadcast cos/sin."""
    r9 = _compose(angles, NUM_STAGES - 1)  # block-diagonal: two 512x512
    w9 = np.concatenate([r9[:HALF, :HALF], r9[HALF:, HALF:]], axis=0)
    a9 = np.asarray(angles, dtype=np.float64)[NUM_STAGES - 1]
    c = np.cos(a9).astype(np.float32)
    s = np.sin(a9).astype(np.float32)
    cs = np.concatenate(
        [
            np.broadcast_to(c, (128, HALF)),
            np.broadcast_to(s, (128, HALF)),
        ],
        axis=0,
    )
    return {
        "w": np.ascontiguousarray(w9, dtype=np.float32),
        "cs": np.ascontiguousarray(cs, dtype=np.float32),
        "ident": np.eye(128, dtype=np.float32),
    }


def build_bass(reps: int = 1):
    """reps>1 repeats the whole pipeline in one NEFF (for marginal timing)."""
    nc = bacc.Bacc(None, target_bir_lowering=False)
    x = nc.dram_tensor("x", [TOK_PER_CORE, DIM], F32, kind="ExternalInput")
    w = nc.dram_tensor("w", [DIM, HALF], F32, kind="ExternalInput")
    cs = nc.dram_tensor("cs", [256, HALF], F32, kind="ExternalInput")
    ident = nc.dram_tensor("ident", [128, 128], F32, kind="ExternalInput")
    y = nc.dram_tensor("y", [TOK_PER_CORE, DIM], F32, kind="ExternalOutput")

    n_sub = TOK_PER_CORE // SUB  # 16 subtiles of 128 tokens

    # Uniform 4-subtile (2 MiB) chunks with a p-major HBM mapping: HBM row
    # r0 + p*4 + s lands on partition p, segment s, so each partition line
    # is one 16 KiB contiguous HBM segment (4 KiB descriptors of the
    # row-interleaved mapping cap out at ~317 GB/s; this reaches ~356).
    # The same permutation is applied on load and store, and each 128-row
    # "subtile" is still a full [128 tok, 1024 feat] block (rows are
    # independent), so compute is unchanged and the permutation cancels.
    # in/out chunkings MUST match for that cancellation.
    in_chunks = [4, 4, 4, 4]
    out_chunks = [4, 4, 4, 4]
    assert sum(in_chunks) == n_sub and sum(out_chunks) == n_sub
    in_start = [sum(in_chunks[:i]) for i in range(len(in_chunks))]
    out_start = [sum(out_chunks[:i]) for i in range(len(out_chunks))]
    sub_to_in_chunk = {}
    for ci, (st, ln) in enumerate(zip(in_start, in_chunks)):
        for s in range(st, st + ln):
            sub_to_in_chunk[s] = ci
    sub_to_out_chunk = {}
    for ci, (st, ln) in enumerate(zip(out_start, out_chunks)):
        for s in range(st, st + ln):
            sub_to_out_chunk[s] = ci

    with tile.TileContext(nc) as tc:
        with (
            tc.tile_pool(name="const", bufs=1) as const_pool,
            tc.tile_pool(name="wstage", bufs=3) as wstage_pool,
            tc.tile_pool(name="xin", bufs=4) as xin_pool,
            tc.tile_pool(name="xt", bufs=5) as xt_pool,
            tc.tile_pool(name="tmp", bufs=8) as tmp_pool,
            tc.tile_pool(name="yout", bufs=3) as yout_pool,
            tc.tile_pool(name="pst", bufs=4, space="PSUM") as pst_pool,
            tc.tile_pool(name="psy", bufs=4, space="PSUM") as psy_pool,
        ):
            # identity goes via the SWDGE ring; the SP ring starts with the
            # first x chunk; W streams in behind it.
            ident_sb = const_pool.tile([128, 128], F32, name="ident_sb")
            nc.gpsimd.dma_start(ident_sb[:], ident[:])

            c_sb = const_pool.tile([128, HALF], F32, name="c_sb")
            s_sb = const_pool.tile([128, HALF], F32, name="s_sb")
            nc.gpsimd.dma_start(c_sb[:], cs[0:128, :])
            nc.gpsimd.dma_start(s_sb[:], cs[128:256, :])

            x_tiles = [None] * len(in_chunks)  # chunk idx -> tile
            y_tiles = [None] * len(out_chunks)

            def load_chunk(ci):
                st, ln = in_start[ci], in_chunks[ci]
                x_tile = xin_pool.tile([128, ln * DIM], F32, name="x_chunk",
                                       tag="x_chunk",
                                       padded_shape=[128, 4 * DIM])
                r0 = st * SUB
                nc.sync.dma_start(
                    x_tile[:, : ln * DIM].rearrange("p (s c) -> p s c", c=DIM),
                    x[r0 : r0 + ln * SUB, :].rearrange("(p s) c -> p s c", s=ln),
                )
                x_tiles[ci] = x_tile

            load_chunk(0)
            first_load_done = True

            # W: DMA the 8 stacked [128, 512] blocks of R9 on the SP ring,
            # then round fp32 -> f32r on DVE (walrus requires f32r matmul
            # inputs to come from a rounding instruction).
            w_sbr = const_pool.tile([128, KB * HALF], F32R, name="w_sbr")
            for kb in range(KB):
                w_stage = wstage_pool.tile([128, HALF], F32, name="w_stage",
                                           tag="w_stage")
                nc.sync.dma_start(
                    w_stage[:], w[kb * 128 : (kb + 1) * 128, :]
                )
                off = kb * HALF
                nc.vector.tensor_copy(w_sbr[:, off : off + HALF], w_stage[:])

            xts = [None] * n_sub

            def emit_transpose(s):
                ci = sub_to_in_chunk[s]
                xcol = (s - in_start[ci]) * DIM
                x_tile = x_tiles[ci]
                ps_t0 = pst_pool.tile([128, 512], F32, name="ps_t0", tag="ps_t")
                ps_t1 = pst_pool.tile([128, 512], F32, name="ps_t1", tag="ps_t")
                for kb in range(KB):
                    dst = ps_t0 if kb < 4 else ps_t1
                    j = (kb % 4) * 128
                    nc.tensor.transpose(
                        dst[:, j : j + 128],
                        x_tile[:, xcol + kb * 128 : xcol + (kb + 1) * 128],
                        ident_sb,
                    )
                xt = xt_pool.tile([128, DIM], F32R, name="xt", tag="xt")
                nc.scalar.copy(xt[:, :512], ps_t0[:])
                nc.scalar.copy(xt[:, 512:], ps_t1[:])
                xts[s] = xt

            def emit_matmul(s):
                co = sub_to_out_chunk[s]
                st, ln = out_start[co], out_chunks[co]
                if s == st:
                    y_tiles[co] = yout_pool.tile(
                        [128, ln * DIM], F32, name="y_chunk", tag="y_chunk",
                        padded_shape=[128, 4 * DIM],
                    )
                y_tile = y_tiles[co]
                ycol = (s - st) * DIM
                xt = xts[s]
                ps_l = psy_pool.tile([128, HALF], F32, name="ps_l", tag="ps_y")
                ps_r = psy_pool.tile([128, HALF], F32, name="ps_r", tag="ps_y")
                for kb in range(4):
                    nc.tensor.matmul(
                        ps_l[:],
                        xt[:, kb * 128 : (kb + 1) * 128],
                        w_sbr[:, kb * HALF : (kb + 1) * HALF],
                        start=(kb == 0),
                        stop=(kb == 3),
                    )
                for kb in range(4, 8):
                    nc.tensor.matmul(
                        ps_r[:],
                        xt[:, kb * 128 : (kb + 1) * 128],
                        w_sbr[:, kb * HALF : (kb + 1) * HALF],
                        start=(kb == 4),
                        stop=(kb == 7),
                    )
                # stage 10: y_l = c*z_l + s*z_r, y_r = c*z_r - s*z_l.
                # DVE does the 4 PSUM-reading multiplies, GpSimd the
                # SBUF-only combines straight into the y chunk tile.
                t1 = tmp_pool.tile([128, HALF], F32, name="t1", tag="tmp")
                u1 = tmp_pool.tile([128, HALF], F32, name="u1", tag="tmp")
                t2 = tmp_pool.tile([128, HALF], F32, name="t2", tag="tmp")
                u2 = tmp_pool.tile([128, HALF], F32, name="u2", tag="tmp")
                nc.vector.tensor_tensor(t1[:], ps_l[:], c_sb[:], MULT)
                nc.vector.tensor_tensor(u1[:], ps_r[:], s_sb[:], MULT)
                nc.vector.tensor_tensor(u2[:], ps_l[:], s_sb[:], MULT)
                nc.vector.tensor_tensor(t2[:], ps_r[:], c_sb[:], MULT)
                nc.gpsimd.tensor_tensor(
                    y_tile[:, ycol : ycol + HALF], t1[:], u1[:], ADD
                )
                nc.gpsimd.tensor_tensor(
                    y_tile[:, ycol + HALF : ycol + DIM], t2[:], u2[:], SUBTRACT
                )
                if s == st + ln - 1:
                    r0 = st * SUB
                    # y stores go out on the ACT HWDGE ring so they don't
                    # queue ahead of later x loads on the SP ring.  Same
                    # p-major permutation as the load, so it cancels.
                    nc.scalar.dma_start(
                        y[r0 : r0 + ln * SUB, :].rearrange("(p s) c -> p s c", s=ln),
                        y_tile[:, : ln * DIM].rearrange("p (s c) -> p s c", c=DIM),
                    )

            # Skewed software pipeline: transposes run two subtiles ahead of
            # the matmuls so the PE never waits on the ScalarE PSUM->SBUF
            # evacuation of its own transpose outputs.
            # All of a rep's x loads are emitted up front: the SP ring
            # stays maximally fed and the tile pool's buffer recycling
            # provides the back-pressure throttle.
            SKEW = 2
            for _rep in range(reps):
                start_ci = 1 if first_load_done else 0
                first_load_done = False
                for ci in range(start_ci, len(in_chunks)):
                    load_chunk(ci)
                for p in range(min(SKEW, n_sub)):
                    emit_transpose(p)
                for s in range(n_sub):
                    nxt = s + SKEW
                    if nxt < n_sub:
                        emit_transpose(nxt)
                    emit_matmul(s)
    nc.compile()
    return nc


_NC_CACHE = None


def _get_nc():
    global _NC_CACHE
    if _NC_CACHE is None:
        _NC_CACHE = build_bass()
    return _NC_CACHE


def run(x: np.ndarray, angles: np.ndarray, trace: bool = False):
    """Run on 8 cores; returns (y_full, BassKernelResults)."""
    x = np.ascontiguousarray(np.asarray(x, dtype=np.float32))
    consts = host_inputs(angles)
    nc = _get_nc()
    in_maps = []
    for c in range(N_CORES):
        in_maps.append(
            {"x": x[c * TOK_PER_CORE : (c + 1) * TOK_PER_CORE], **consts}
        )
    res = run_bass_kernel_spmd(
        nc, in_maps, core_ids=list(range(N_CORES)), trace=trace
    )
    y = np.concatenate([res.results[c]["y"] for c in range(N_CORES)], axis=0)
    return y, res


def kernel(x: np.ndarray, angles: np.ndarray) -> np.ndarray:
    y, _ = run(x, angles, trace=False)
    return y


# revision 12
# speedup vs baseline: 1.8547x; 1.0349x over previous
"""Butterfly permuter kernel for Trainium2 (8 NeuronCores, SPMD data-parallel).

The reference applies 10 butterfly rotation stages along the feature axis
(dim=1024) of x [16384, 1024].  Stages 1-9 act within the two 512-wide
feature halves, so they compose into a block-diagonal matrix
R9 = blockdiag(A0, A1) with two dense 512x512 blocks (computed on the host
in float64).  Stage 10 couples feature f with f+512 through an elementwise
Givens rotation whose cos/sin vary along the feature axis.

Per core (x shard [2048, 1024], 16 subtiles of 128 tokens):
  - DMA x in 2 MiB chunks with a p-major row permutation (16 KiB
    contiguous HBM segment per partition line; the row-interleaved
    mapping's 4 KiB descriptors cap at ~317 GB/s, this reaches ~356 of
    the ~358 GB/s HBM-per-NC limit).  The same permutation is used on the
    y store, so it cancels; rows are independent so compute never sees it.
  - PE-transpose each [128 tok, 128 feat] block via identity (PSUM),
    evacuate to SBUF on ScalarE as f32r
  - z halves: ps_l = sum_{kb<4} xt_kb^T @ A0_kb, ps_r = sum_{kb>=4} ... -
    8 accumulating f32r matmuls of N=512 per subtile (half the PE work of
    the dense 1024x1024 formulation)
  - stage 10 fused into PSUM evacuation: 4 DVE multiplies against
    broadcast cos/sin tiles + 2 GpSimd add/sub produce
    y_l = c*z_l + s*z_r, y_r = c*z_r - s*z_l directly in the y SBUF tile
  - DMA y out in 2 MiB chunks on the ACT ring

Engine budget per rep per core: PE ~34-41 us, DVE ~17 us, ScalarE ~14 us,
GpSimd ~14 us, DMA 16 MiB @ ~356 GB/s ~47 us -> memory-bound as targeted
(measured 49.4 us/rep steady state vs 47.0 us for the bare DMA pipeline).
"""

import numpy as np

import concourse.bass as bass
import concourse.mybir as mybir
import concourse.tile as tile
from concourse import bacc
from concourse.bass_utils import run_bass_kernel_spmd

N_CORES = 8
DIM = 1024
HALF = DIM // 2
NUM_STAGES = 10
N_TOKENS = 16384
TOK_PER_CORE = N_TOKENS // N_CORES  # 2048
SUB = 128  # tokens per subtile (partition dim)
KB = DIM // 128  # 8 feature blocks

F32 = mybir.dt.float32
F32R = mybir.dt.float32r
MULT = mybir.AluOpType.mult
ADD = mybir.AluOpType.add
SUBTRACT = mybir.AluOpType.subtract


def _compose(angles: np.ndarray, n_stages: int) -> np.ndarray:
    """Compose the first n_stages butterfly stages: y = x @ R."""
    y = np.eye(DIM, dtype=np.float64)
    a = np.asarray(angles, dtype=np.float64)
    for s in range(n_stages):
        span = 2 ** (s + 1)
        half = span // 2
        y = y.reshape(-1, DIM // span, span)
        left, right = y[..., :half], y[..., half:]
        th = a[s].reshape(1, DIM // span, half)
        c, sn = np.cos(th), np.sin(th)
        y = np.concatenate([c * left + sn * right, -sn * left + c * right], -1)
        y = y.reshape(-1, DIM)
    return y


def host_inputs(angles: np.ndarray) -> dict:
    """Per-core constant inputs: stacked R9 blocks + broadcast cos/sin."""
    r9 = _compose(angles, NUM_STAGES - 1)  # block-diagonal: two 512x512
    w9 = np.concatenate([r9[:HALF, :HALF], r9[HALF:, HALF:]], axis=0)
    a9 = np.asarray(angles, dtype=np.float64)[NUM_STAGES - 1]
    c = np.cos(a9).astype(np.float32)
    s = np.sin(a9).astype(np.float32)
    cs = np.concatenate(
        [
            np.broadcast_to(c, (128, HALF)),
            np.broadcast_to(s, (128, HALF)),
        ],
        axis=0,
    )
    return {
        "w": np.ascontiguousarray(w9, dtype=np.float32),
        "cs": np.ascontiguousarray(cs, dtype=np.float32),
        "ident": np.eye(128, dtype=np.float32),
    }


def build_bass(reps: int = 1):
    """reps>1 repeats the whole pipeline in one NEFF (for marginal timing)."""
    nc = bacc.Bacc(None, target_bir_lowering=False)
    x = nc.dram_tensor("x", [TOK_PER_CORE, DIM], F32, kind="ExternalInput")
    w = nc.dram_tensor("w", [DIM, HALF], F32, kind="ExternalInput")
    cs = nc.dram_tensor("cs", [256, HALF], F32, kind="ExternalInput")
    ident = nc.dram_tensor("ident", [128, 128], F32, kind="ExternalInput")
    y = nc.dram_tensor("y", [TOK_PER_CORE, DIM], F32, kind="ExternalOutput")

    n_sub = TOK_PER_CORE // SUB  # 16 subtiles of 128 tokens

    # Uniform 4-subtile (2 MiB) chunks with a p-major HBM mapping: HBM row
    # r0 + p*4 + s lands on partition p, segment s, so each partition line
    # is one 16 KiB contiguous HBM segment (4 KiB descriptors of the
    # row-interleaved mapping cap out at ~317 GB/s; this reaches ~356).
    # The same permutation is applied on load and store, and each 128-row
    # "subtile" is still a full [128 tok, 1024 feat] block (rows are
    # independent), so compute is unchanged and the permutation cancels.
    # in/out chunkings MUST match for that cancellation.
    in_chunks = [4, 4, 4, 4]
    out_chunks = [4, 4, 4, 4]
    assert sum(in_chunks) == n_sub and sum(out_chunks) == n_sub
    in_start = [sum(in_chunks[:i]) for i in range(len(in_chunks))]
    out_start = [sum(out_chunks[:i]) for i in range(len(out_chunks))]
    sub_to_in_chunk = {}
    for ci, (st, ln) in enumerate(zip(in_start, in_chunks)):
        for s in range(st, st + ln):
            sub_to_in_chunk[s] = ci
    sub_to_out_chunk = {}
    for ci, (st, ln) in enumerate(zip(out_start, out_chunks)):
        for s in range(st, st + ln):
            sub_to_out_chunk[s] = ci

    with tile.TileContext(nc) as tc:
        with (
            tc.tile_pool(name="const", bufs=1) as const_pool,
            tc.tile_pool(name="wstage", bufs=3) as wstage_pool,
            tc.tile_pool(name="xin", bufs=4) as xin_pool,
            tc.tile_pool(name="xt", bufs=5) as xt_pool,
            tc.tile_pool(name="tmp", bufs=8) as tmp_pool,
            tc.tile_pool(name="yout", bufs=4) as yout_pool,
            tc.tile_pool(name="pst", bufs=4, space="PSUM") as pst_pool,
            tc.tile_pool(name="psy", bufs=4, space="PSUM") as psy_pool,
        ):
            # identity goes via the SWDGE ring; the SP ring starts with the
            # first x chunk; W streams in behind it.
            ident_sb = const_pool.tile([128, 128], F32, name="ident_sb")
            nc.gpsimd.dma_start(ident_sb[:], ident[:])

            c_sb = const_pool.tile([128, HALF], F32, name="c_sb")
            s_sb = const_pool.tile([128, HALF], F32, name="s_sb")
            nc.gpsimd.dma_start(c_sb[:], cs[0:128, :])
            nc.gpsimd.dma_start(s_sb[:], cs[128:256, :])

            x_tiles = [None] * len(in_chunks)  # chunk idx -> tile
            y_tiles = [None] * len(out_chunks)

            def load_chunk(ci):
                st, ln = in_start[ci], in_chunks[ci]
                x_tile = xin_pool.tile([128, ln * DIM], F32, name="x_chunk",
                                       tag="x_chunk",
                                       padded_shape=[128, 4 * DIM])
                r0 = st * SUB
                nc.sync.dma_start(
                    x_tile[:, : ln * DIM].rearrange("p (s c) -> p s c", c=DIM),
                    x[r0 : r0 + ln * SUB, :].rearrange("(p s) c -> p s c", s=ln),
                )
                x_tiles[ci] = x_tile

            load_chunk(0)
            first_load_done = True

            # W: DMA the 8 stacked [128, 512] blocks of R9 on the SWDGE
            # ring (so it doesn't queue behind the first x chunks on SP),
            # then round fp32 -> f32r on DVE (walrus requires f32r matmul
            # inputs to come from a rounding instruction).
            w_sbr = const_pool.tile([128, KB * HALF], F32R, name="w_sbr")
            for kb in range(KB):
                w_stage = wstage_pool.tile([128, HALF], F32, name="w_stage",
                                           tag="w_stage")
                nc.gpsimd.dma_start(
                    w_stage[:], w[kb * 128 : (kb + 1) * 128, :]
                )
                off = kb * HALF
                nc.vector.tensor_copy(w_sbr[:, off : off + HALF], w_stage[:])

            xts = [None] * n_sub

            def emit_transpose(s):
                ci = sub_to_in_chunk[s]
                xcol = (s - in_start[ci]) * DIM
                x_tile = x_tiles[ci]
                ps_t0 = pst_pool.tile([128, 512], F32, name="ps_t0", tag="ps_t")
                ps_t1 = pst_pool.tile([128, 512], F32, name="ps_t1", tag="ps_t")
                for kb in range(KB):
                    dst = ps_t0 if kb < 4 else ps_t1
                    j = (kb % 4) * 128
                    nc.tensor.transpose(
                        dst[:, j : j + 128],
                        x_tile[:, xcol + kb * 128 : xcol + (kb + 1) * 128],
                        ident_sb,
                    )
                xt = xt_pool.tile([128, DIM], F32R, name="xt", tag="xt")
                nc.scalar.copy(xt[:, :512], ps_t0[:])
                nc.scalar.copy(xt[:, 512:], ps_t1[:])
                xts[s] = xt

            def emit_matmul(s):
                co = sub_to_out_chunk[s]
                st, ln = out_start[co], out_chunks[co]
                if s == st:
                    y_tiles[co] = yout_pool.tile(
                        [128, ln * DIM], F32, name="y_chunk", tag="y_chunk",
                        padded_shape=[128, 4 * DIM],
                    )
                y_tile = y_tiles[co]
                ycol = (s - st) * DIM
                xt = xts[s]
                ps_l = psy_pool.tile([128, HALF], F32, name="ps_l", tag="ps_y")
                ps_r = psy_pool.tile([128, HALF], F32, name="ps_r", tag="ps_y")
                for kb in range(4):
                    nc.tensor.matmul(
                        ps_l[:],
                        xt[:, kb * 128 : (kb + 1) * 128],
                        w_sbr[:, kb * HALF : (kb + 1) * HALF],
                        start=(kb == 0),
                        stop=(kb == 3),
                    )
                for kb in range(4, 8):
                    nc.tensor.matmul(
                        ps_r[:],
                        xt[:, kb * 128 : (kb + 1) * 128],
                        w_sbr[:, kb * HALF : (kb + 1) * HALF],
                        start=(kb == 4),
                        stop=(kb == 7),
                    )
                # stage 10: y_l = c*z_l + s*z_r, y_r = c*z_r - s*z_l.
                # DVE does the 4 PSUM-reading multiplies, GpSimd the
                # SBUF-only combines straight into the y chunk tile.
                t1 = tmp_pool.tile([128, HALF], F32, name="t1", tag="tmp")
                u1 = tmp_pool.tile([128, HALF], F32, name="u1", tag="tmp")
                t2 = tmp_pool.tile([128, HALF], F32, name="t2", tag="tmp")
                u2 = tmp_pool.tile([128, HALF], F32, name="u2", tag="tmp")
                nc.vector.tensor_tensor(t1[:], ps_l[:], c_sb[:], MULT)
                nc.vector.tensor_tensor(u1[:], ps_r[:], s_sb[:], MULT)
                nc.vector.tensor_tensor(u2[:], ps_l[:], s_sb[:], MULT)
                nc.vector.tensor_tensor(t2[:], ps_r[:], c_sb[:], MULT)
                nc.gpsimd.tensor_tensor(
                    y_tile[:, ycol : ycol + HALF], t1[:], u1[:], ADD
                )
                nc.gpsimd.tensor_tensor(
                    y_tile[:, ycol + HALF : ycol + DIM], t2[:], u2[:], SUBTRACT
                )
                if s == st + ln - 1:
                    r0 = st * SUB
                    # y stores go out on the ACT HWDGE ring so they don't
                    # queue ahead of later x loads on the SP ring.  Same
                    # p-major permutation as the load, so it cancels.
                    nc.scalar.dma_start(
                        y[r0 : r0 + ln * SUB, :].rearrange("(p s) c -> p s c", s=ln),
                        y_tile[:, : ln * DIM].rearrange("p (s c) -> p s c", c=DIM),
                    )

            # Skewed software pipeline: transposes run two subtiles ahead of
            # the matmuls so the PE never waits on the ScalarE PSUM->SBUF
            # evacuation of its own transpose outputs.
            # All of a rep's x loads are emitted up front: the SP ring
            # stays maximally fed and the tile pool's buffer recycling
            # provides the back-pressure throttle.
            SKEW = 2
            for _rep in range(reps):
                start_ci = 1 if first_load_done else 0
                first_load_done = False
                for ci in range(start_ci, len(in_chunks)):
                    load_chunk(ci)
                for p in range(min(SKEW, n_sub)):
                    emit_transpose(p)
                for s in range(n_sub):
                    nxt = s + SKEW
                    if nxt < n_sub:
                        emit_transpose(nxt)
                    emit_matmul(s)
    nc.compile()
    return nc


_NC_CACHE = None


def _get_nc():
    global _NC_CACHE
    if _NC_CACHE is None:
        _NC_CACHE = build_bass()
    return _NC_CACHE


def run(x: np.ndarray, angles: np.ndarray, trace: bool = False):
    """Run on 8 cores; returns (y_full, BassKernelResults)."""
    x = np.ascontiguousarray(np.asarray(x, dtype=np.float32))
    consts = host_inputs(angles)
    nc = _get_nc()
    in_maps = []
    for c in range(N_CORES):
        in_maps.append(
            {"x": x[c * TOK_PER_CORE : (c + 1) * TOK_PER_CORE], **consts}
        )
    res = run_bass_kernel_spmd(
        nc, in_maps, core_ids=list(range(N_CORES)), trace=trace
    )
    y = np.concatenate([res.results[c]["y"] for c in range(N_CORES)], axis=0)
    return y, res


def kernel(x: np.ndarray, angles: np.ndarray) -> np.ndarray:
    y, _ = run(x, angles, trace=False)
    return y
